# revision 1
# baseline (speedup 1.0000x reference)
"""Trainium2 Bass kernel for nn_R_GAMLP_RLU (GAMLP recursive-label-utilization head).

Strategy:
  - Shard the node dimension N=100000 across 8 NeuronCores (12500 nodes each);
    no device-to-device communication (host gathers per-core logits).
  - Per core, process node tiles of T=500 (4 subtiles of 125 partitions).
  - Attention recurrence is computed via the linearity trick:
        history_i @ wl == sum_h att[h] * (feat_h @ wl)
    so only per-hop projections xl/xr (PE matmuls) and tiny [125,4] vector ops
    are needed for the score recurrence; features are then combined once with
    the final hop attention (unnormalized exp weights, normalized at the end).
  - MLPs run on PE in bf16 with channels-on-partitions layout.
  - Inputs are uploaded in bf16 in both node-major (weighted sum) and
    feature-major (projection rhs) tiled layouts, pre-arranged on host so every
    DMA is a large contiguous-per-partition transfer.
"""

import sys

if "/opt/trn_rl_repo" not in sys.path:
    sys.path.insert(0, "/opt/trn_rl_repo")

import numpy as np
import ml_dtypes

import bass_rust
import concourse.bass as bass
import concourse.mybir as mybir
from concourse import tile
from concourse.bass_utils import run_bass_kernel_spmd

BF16 = mybir.dt.bfloat16
F32 = mybir.dt.float32
bfnp = ml_dtypes.bfloat16
OP = mybir.AluOpType
AF = mybir.ActivationFunctionType

H, F, HID, C = 10, 256, 512, 47
N_FULL = 100000
N_CORES = 8
NPC = N_FULL // N_CORES  # 12500
T = 500                  # nodes per tile
SUB = 125                # nodes per subtile (partition dim)
NS = T // SUB            # 4
NT_FULL = NPC // T       # 25
LEAK = 0.2               # leaky_relu slope of the attention act
GCN_ALPHA = 0.5          # GCNII alpha

_TC = tile.TileContext

_WAIT_CAP = 1  # this walrus build rejects >1 sync wait command per instruction


def _split_sync_waits(nc):
    """Hoist excess per-instruction sem waits onto same-engine
    InstEventSemaphore carriers inserted immediately before the instruction
    (walrus here caps sync waits at 1 per instruction)."""
    n = 0
    for fn in nc.m.functions:
        for bb in fn.blocks:
            insts = bb.instructions
            i = 0
            while i < len(insts):
                inst = insts[i]
                si = inst.sync_info
                waits = list(si.on_wait) if si else []
                if len(waits) > _WAIT_CAP:
                    upd = list(si.on_update) if si else []
                    extra, keep = waits[:-_WAIT_CAP], waits[-_WAIT_CAP:]
                    carriers = []
                    for k in range(0, len(extra), _WAIT_CAP):
                        nop = mybir.InstEventSemaphore(
                            name=f"wsplit_{n}", ins=[], outs=[]
                        )
                        n += 1
                        nop.engine = inst.engine
                        nop.sync_info = bass_rust.SyncInfo(
                            on_wait=extra[k : k + _WAIT_CAP], on_update=[]
                        )
                        nc.register_instruction(nop, overwrite=True)
                        carriers.append(nop)
                    inst.sync_info = bass_rust.SyncInfo(on_wait=keep, on_update=upd)
                    insts[i:i] = carriers
                    i += len(carriers)
                i += 1
    return n


def build(n_tiles, ba, a_out, a_lab, repeat=1):
    nc = bass.Bass()

    featN = nc.dram_tensor("featN", [n_tiles, SUB, H, NS, F], BF16, kind="ExternalInput")
    featT = nc.dram_tensor("featT", [n_tiles, 128, H, 2, T], BF16, kind="ExternalInput")
    embT = nc.dram_tensor("embT", [n_tiles, C, T], BF16, kind="ExternalInput")
    w2d = nc.dram_tensor("w2", [128, 2, 32], BF16, kind="ExternalInput")
    w0d = nc.dram_tensor("w0", [F, HID], BF16, kind="ExternalInput")
    wg1d = nc.dram_tensor("wg1", [HID, HID], BF16, kind="ExternalInput")
    wg2d = nc.dram_tensor("wg2", [HID, HID], BF16, kind="ExternalInput")
    wlastd = nc.dram_tensor("wlast", [HID, C], BF16, kind="ExternalInput")
    wl0d = nc.dram_tensor("wl0", [C, HID], BF16, kind="ExternalInput")
    wl1d = nc.dram_tensor("wl1", [HID, HID], BF16, kind="ExternalInput")
    wl2d = nc.dram_tensor("wl2", [HID, HID], BF16, kind="ExternalInput")
    wl3d = nc.dram_tensor("wl3", [HID, C], BF16, kind="ExternalInput")
    b0d = nc.dram_tensor("b0t", [128, 4], F32, kind="ExternalInput")
    b0hd = nc.dram_tensor("b0h", [128, 4], F32, kind="ExternalInput")
    bl0d = nc.dram_tensor("bl0t", [128, 4], F32, kind="ExternalInput")
    bl1d = nc.dram_tensor("bl1t", [128, 4], F32, kind="ExternalInput")
    bl2d = nc.dram_tensor("bl2t", [128, 4], F32, kind="ExternalInput")
    bfind = nc.dram_tensor("bfin", [C, 1], F32, kind="ExternalInput")
    idfd = nc.dram_tensor("idf", [128, 128], F32, kind="ExternalInput")
    idbd = nc.dram_tensor("idb", [128, 128], BF16, kind="ExternalInput")
    idqd = nc.dram_tensor("idq", [128, 4], F32, kind="ExternalInput")

    outd = nc.dram_tensor("outT", [n_tiles, SUB, NS, C], F32, kind="ExternalOutput")

    with _TC(nc) as tc:
        with (
            tc.tile_pool(name="consts", bufs=1) as cp,
            tc.tile_pool(name="feat", bufs=2) as fp,
            tc.tile_pool(name="act", bufs=1) as ap,
            tc.tile_pool(name="small", bufs=2) as sp,
            tc.tile_pool(name="ps", bufs=1, space="PSUM") as ps,
        ):
            # ---- constants ----
            idf = cp.tile([128, 128], F32)
            idb = cp.tile([128, 128], BF16)
            idq = cp.tile([128, 4], F32)
            w2 = cp.tile([128, 2, 32], BF16)
            w0 = cp.tile([128, 2, HID], BF16)
            wg1 = cp.tile([128, 4, HID], BF16)
            wg2 = cp.tile([128, 4, HID], BF16)
            wlast = cp.tile([128, 4, C], BF16)
            wl0 = cp.tile([C, HID], BF16)
            wl1 = cp.tile([128, 4, HID], BF16)
            wl2 = cp.tile([128, 4, HID], BF16)
            wl3 = cp.tile([128, 4, C], BF16)
            b0 = cp.tile([128, 4], F32)
            b0h = cp.tile([128, 4], F32)
            bl0 = cp.tile([128, 4], F32)
            bl1 = cp.tile([128, 4], F32)
            bl2 = cp.tile([128, 4], F32)
            bfin = cp.tile([C, 1], F32)

            nc.sync.dma_start(idf[:], idfd[:])
            nc.sync.dma_start(idb[:], idbd[:])
            nc.sync.dma_start(idq[:], idqd[:])
            nc.sync.dma_start(w2[:], w2d[:])
            nc.sync.dma_start(w0[:], w0d.rearrange("(c p) m -> p c m", p=128))
            nc.sync.dma_start(wg1[:], wg1d.rearrange("(c p) m -> p c m", p=128))
            nc.sync.dma_start(wg2[:], wg2d.rearrange("(c p) m -> p c m", p=128))
            nc.sync.dma_start(wlast[:], wlastd.rearrange("(c p) m -> p c m", p=128))
            nc.sync.dma_start(wl0[:], wl0d[:])
            nc.sync.dma_start(wl1[:], wl1d.rearrange("(c p) m -> p c m", p=128))
            nc.sync.dma_start(wl2[:], wl2d.rearrange("(c p) m -> p c m", p=128))
            nc.sync.dma_start(wl3[:], wl3d.rearrange("(c p) m -> p c m", p=128))
            nc.sync.dma_start(b0[:], b0d[:])
            nc.sync.dma_start(b0h[:], b0hd[:])
            nc.sync.dma_start(bl0[:], bl0d[:])
            nc.sync.dma_start(bl1[:], bl1d[:])
            nc.sync.dma_start(bl2[:], bl2d[:])
            nc.sync.dma_start(bfin[:], bfind[:])

            def front(t):
                """Loads + projections + attention recurrence + weighted sum."""
                fN = fp.tile([SUB, H, NS, F], BF16, tag="fN", name=f"fN_{t}")
                fT = fp.tile([128, H, 2, T], BF16, tag="fT", bufs=3, name=f"fT_{t}")
                emb = fp.tile([C, T], BF16, tag="emb", bufs=3, name=f"emb_{t}")
                for k in range(5):
                    nc.gpsimd.dma_start(
                        fN[:, 2 * k : 2 * k + 2], featN[t, :, 2 * k : 2 * k + 2]
                    )
                    nc.sync.dma_start(
                        fT[:, 2 * k : 2 * k + 2], featT[t, :, 2 * k : 2 * k + 2]
                    )
                nc.sync.dma_start(emb[:], embT[t])

                # hop projections xl/xr: hop h -> rows 32*(h%4)..+2 of bank
                # h//4 (hops 0-7 in pp, hops 8-9 in a second use pp2)
                pp = ps.tile([128, 2, 512], F32, tag="proj", bufs=1, name=f"pp_{t}")
                for h in range(8):
                    r = 32 * (h % 4)
                    for c in range(2):
                        nc.tensor.matmul(
                            pp[r : r + 32, h // 4, :T],
                            w2[:, c, :],
                            fT[:, h, c, :],
                            start=(c == 0),
                            stop=(c == 1),
                            tile_position=(0, r),
                        )
                xlxr = ap.tile([128, 3, T], F32, tag="xlxr", bufs=2, name=f"xlxr_{t}")
                nc.vector.tensor_copy(xlxr[:, 0, :], pp[:, 0, :T])
                nc.vector.tensor_copy(xlxr[:, 1, :], pp[:, 1, :T])
                pp2 = ps.tile([128, 2, 512], F32, tag="proj", bufs=1, name=f"pp2_{t}")
                for h in range(8, H):
                    r = 32 * (h % 4)
                    for c in range(2):
                        nc.tensor.matmul(
                            pp2[r : r + 32, 0, :T],
                            w2[:, c, :],
                            fT[:, h, c, :],
                            start=(c == 0),
                            stop=(c == 1),
                            tile_position=(0, r),
                        )
                nc.vector.tensor_copy(xlxr[:64, 2, :], pp2[:64, 0, :T])

                # transpose pairs to node-major: px[:, s, 2h:2h+2]
                px = ps.tile([SUB, NS, 128], F32, tag="px", bufs=1, name=f"px_{t}")
                for h in range(H):
                    r = 32 * (h % 4)
                    b = h // 4 if h < 8 else 2
                    for s in range(NS):
                        nc.tensor.transpose(
                            px[:, s, 2 * h : 2 * h + 2],
                            xlxr[r : r + 2, b, s * SUB : (s + 1) * SUB],
                            idq[r : r + 2, :2],
                            tile_position=(r, 0),
                        )
                xn = sp.tile([SUB, NS, 2 * H], F32, tag="xn", name=f"xn_{t}")
                nc.vector.tensor_copy(xn[:], px[:, :, : 2 * H])

                # attention recurrence (f32, [SUB, NS] ops)
                sc = sp.tile([SUB, NS, H], F32, tag="sc", name=f"sc_{t}")
                ex = sp.tile([SUB, NS, H], F32, tag="ex", name=f"ex_{t}")
                num = sp.tile([SUB, NS], F32, tag="num", name=f"num_{t}")
                den = sp.tile([SUB, NS], F32, tag="den", name=f"den_{t}")
                tmp = sp.tile([SUB, NS], F32, tag="tmp", name=f"tmp_{t}")
                tmp2 = sp.tile([SUB, NS], F32, tag="tmp2", name=f"tmp2_{t}")
                z = sp.tile([SUB, NS], F32, tag="z", name=f"z_{t}")
                rec = sp.tile([SUB, NS], F32, tag="rec", name=f"rec_{t}")

                nc.vector.scalar_tensor_tensor(
                    z[:], xn[:, :, 0], float(ba), xn[:, :, 1], op0=OP.add, op1=OP.add
                )
                nc.vector.scalar_tensor_tensor(
                    sc[:, :, 0], z[:], LEAK, z[:], op0=OP.mult, op1=OP.max
                )
                nc.scalar.activation(ex[:, :, 0], sc[:, :, 0], AF.Exp)
                nc.vector.tensor_copy(den[:], ex[:, :, 0])
                nc.vector.tensor_mul(num[:], ex[:, :, 0], xn[:, :, 0])
                for i in range(1, H):
                    nc.vector.reciprocal(rec[:], den[:])
                    nc.vector.tensor_mul(tmp[:], num[:], rec[:])
                    nc.vector.scalar_tensor_tensor(
                        z[:], tmp[:], float(ba), xn[:, :, 2 * i + 1],
                        op0=OP.add, op1=OP.add,
                    )
                    nc.vector.scalar_tensor_tensor(
                        sc[:, :, i], z[:], LEAK, z[:], op0=OP.mult, op1=OP.max
                    )
                    nc.scalar.activation(ex[:, :, i], sc[:, :, i], AF.Exp)
                    nc.vector.tensor_add(den[:], den[:], ex[:, :, i])
                    if i < H - 1:
                        nc.vector.tensor_mul(tmp2[:], ex[:, :, i], xn[:, :, 2 * i])
                        nc.vector.tensor_add(num[:], num[:], tmp2[:])
                recf = sp.tile([SUB, NS], F32, tag="recf", name=f"recf_{t}")
                nc.vector.reciprocal(recf[:], den[:])

                # weighted feature sum: right[s] = sum_h diag(e_h*recf) @ fN_h
                # via PE matmuls accumulating f32 in PSUM, then transpose.
                pright = ps.tile([SUB, NS, 256], F32, tag="proj", bufs=1,
                                 name=f"pright_{t}")
                for s in range(NS):
                    for h in range(H):
                        dg = ap.tile([SUB, 128], BF16, tag="diag", bufs=4,
                                     name=f"dg_{t}_{s}_{h}")
                        nc.vector.tensor_scalar(
                            dg[:, :SUB], idb[:SUB, :SUB],
                            ex[:, s, h : h + 1], recf[:, s : s + 1],
                            op0=OP.mult, op1=OP.mult,
                        )
                        nc.tensor.matmul(
                            pright[:, s, :], dg[:, :SUB], fN[:, h, s, :],
                            start=(h == 0), stop=(h == H - 1),
                        )
                right = ap.tile([SUB, NS, 256], BF16, tag="right", bufs=2,
                                name=f"right_{t}")
                nc.scalar.copy(right[:], pright[:])
                prT = ps.tile([128, 2, 4, 128], BF16, tag="proj", bufs=1,
                              name=f"prT_{t}")
                for s in range(NS):
                    for c in range(2):
                        nc.tensor.transpose(
                            prT[:, c, s, :SUB],
                            right[:, s, c * 128 : (c + 1) * 128],
                            idb[:SUB, :SUB],
                        )
                rT = ap.tile([128, 2, 4, 128], BF16, tag="rT", bufs=2, name=f"rT_{t}")
                nc.vector.tensor_copy(rT[:, 0, :, :SUB], prT[:, 0, :, :SUB])
                nc.vector.tensor_copy(rT[:, 1, :, :SUB], prT[:, 1, :, :SUB])
                return t, emb, rT

            def backx(state):
                """lr_output MLP (x path) through the w_last accumulation."""
                t, emb, rT = state
                # lr_output MLP (x path)
                h0q = ap.tile([128, 4, T], BF16, tag="h0q", name=f"h0q_{t}")
                xi1 = ap.tile([128, 4, T], BF16, tag="xi", bufs=2, name=f"xi1_{t}")
                for half in range(2):
                    pb = ps.tile([128, 2, 512], F32, tag="big", bufs=2,
                                 name=f"pb_{t}_{half}")
                    for m in range(2):
                        mc = 2 * half + m
                        for c in range(2):
                            nc.tensor.matmul(
                                pb[:, m, :T],
                                w0[:, c, mc * 128 : (mc + 1) * 128],
                                rT[:, c, :, :SUB],
                                start=(c == 0),
                                stop=(c == 1),
                            )
                    for m in range(2):
                        mc = 2 * half + m
                        nc.vector.tensor_scalar(
                            h0q[:, mc, :], pb[:, m, :T],
                            GCN_ALPHA, b0h[:, mc : mc + 1],
                            op0=OP.mult, op1=OP.add,
                        )
                        nc.scalar.activation(
                            xi1[:, mc, :], pb[:, m, :T], AF.Prelu,
                            bias=b0[:, mc : mc + 1], alpha=float(a_out),
                        )

                xi_in = xi1
                for gi, wg in enumerate((wg1, wg2)):
                    sup = ap.tile([128, 4, T], BF16, tag="sup", bufs=2,
                                  name=f"sup_{t}_{gi}")
                    for mc in (0, 2):
                        nc.vector.scalar_tensor_tensor(
                            sup[:, mc : mc + 2, :], xi_in[:, mc : mc + 2, :],
                            1.0 - GCN_ALPHA, h0q[:, mc : mc + 2, :],
                            op0=OP.mult, op1=OP.add,
                        )
                    xi_out = ap.tile([128, 4, T], BF16, tag="xi", bufs=2,
                                     name=f"xi_{t}_{gi}")
                    for half in range(2):
                        pb = ps.tile([128, 2, 512], F32, tag="big", bufs=2,
                                     name=f"pg_{t}_{half}_{gi}")
                        for m in range(2):
                            mc = 2 * half + m
                            for c in range(4):
                                nc.tensor.matmul(
                                    pb[:, m, :T],
                                    wg[:, c, mc * 128 : (mc + 1) * 128],
                                    sup[:, c, :],
                                    start=(c == 0),
                                    stop=False,
                                )
                            nc.tensor.matmul(
                                pb[:, m, :T], idb[:, :], xi_in[:, mc, :],
                                start=False, stop=True,
                            )
                        mc = 2 * half
                        nc.scalar.activation(
                            xi_out[:, mc : mc + 2, :], pb[:, :, :T], AF.Prelu,
                            alpha=float(a_out),
                        )
                    xi_in = xi_out

                po = ps.tile([C, 512], F32, tag="late", name=f"po_{t}")
                for c in range(4):
                    nc.tensor.matmul(
                        po[:, :T], wlast[:, c, :], xi_in[:, c, :],
                        start=(c == 0), stop=False, skip_group_check=True,
                    )

                return t, emb, po

            def backy(state):
                """label_fc (y path) + final bias + output transpose + store."""
                t, emb, po = state
                y_in = None
                for li, (wl, blv) in enumerate(((wl0, bl0), (wl1, bl1), (wl2, bl2))):
                    y_out = ap.tile([128, 4, T], BF16, tag="y", bufs=2,
                                    name=f"y_{t}_{li}")
                    for half in range(2):
                        pb = ps.tile([128, 2, 512], F32, tag="big", bufs=2,
                                     name=f"py_{t}_{half}_{li}")
                        for m in range(2):
                            mc = 2 * half + m
                            if li == 0:
                                nc.tensor.matmul(
                                    pb[:, m, :T], wl0[:, mc * 128 : (mc + 1) * 128],
                                    emb[:], start=True, stop=True,
                                )
                            else:
                                for c in range(4):
                                    nc.tensor.matmul(
                                        pb[:, m, :T],
                                        wl[:, c, mc * 128 : (mc + 1) * 128],
                                        y_in[:, c, :],
                                        start=(c == 0),
                                        stop=(c == 3),
                                    )
                        for m in range(2):
                            mc = 2 * half + m
                            nc.scalar.activation(
                                y_out[:, mc, :], pb[:, m, :T], AF.Prelu,
                                bias=blv[:, mc : mc + 1], alpha=float(a_lab),
                            )
                    y_in = y_out

                for c in range(4):
                    nc.tensor.matmul(
                        po[:, :T], wl3[:, c, :], y_in[:, c, :],
                        start=False, stop=(c == 3), skip_group_check=True,
                    )

                # final bias + output transpose + store
                outx = ap.tile([C, T], F32, tag="outx", bufs=2, name=f"outx_{t}")
                nc.scalar.activation(
                    outx[:], po[:, :T], AF.Identity, bias=bfin[:, 0:1]
                )
                pt = ps.tile([SUB, NS, 128], F32, tag="late", name=f"pt_{t}")
                for s in range(NS):
                    nc.tensor.transpose(
                        pt[:, s, :C],
                        outx[:, s * SUB : (s + 1) * SUB],
                        idf[:C, :C],
                    )
                outT = ap.tile([SUB, NS, C], F32, tag="outT", bufs=2,
                               name=f"outT_{t}")
                nc.vector.tensor_copy(outT[:], pt[:, :, :C])
                nc.gpsimd.dma_start(outd[t], outT[:])

            def whole():
                state = front(0)
                for t in range(1, n_tiles):
                    nxt = front(t)
                    backy(backx(state))
                    state = nxt
                backy(backx(state))

            if repeat == 1:
                whole()
            else:
                with tc.For_i(0, repeat, 1):
                    whole()

    _split_sync_waits(nc)
    return nc


def _prep_weights(inputs, n_tiles):
    """Common (per-core-identical) weight/const arrays for the in_maps."""
    f32 = np.float32
    wa = np.asarray(inputs["wa"], f32)
    wl, wr = wa[:F], wa[F:]
    w2 = np.zeros((128, 2, 32), bfnp)
    w2s = np.stack([wl, wr], axis=1).astype(bfnp)  # [256, 2]
    w2[:, 0, :2] = w2s[:128]
    w2[:, 1, :2] = w2s[128:]
    b0 = np.asarray(inputs["b0"], f32)
    blast = np.asarray(inputs["b_last"], f32) + np.asarray(inputs["bl3"], f32)
    idq = np.zeros((128, 4), f32)
    for j in range(4):
        idq[32 * j : 32 * j + 4, :] = np.eye(4, dtype=f32)
    m = {
        "w2": w2,
        "w0": np.asarray(inputs["w0"]).astype(bfnp),
        "wg1": np.asarray(inputs["wg1"]).astype(bfnp),
        "wg2": np.asarray(inputs["wg2"]).astype(bfnp),
        "wlast": np.asarray(inputs["w_last"]).astype(bfnp),
        "wl0": np.asarray(inputs["wl0"]).astype(bfnp),
        "wl1": np.asarray(inputs["wl1"]).astype(bfnp),
        "wl2": np.asarray(inputs["wl2"]).astype(bfnp),
        "wl3": np.asarray(inputs["wl3"]).astype(bfnp),
        "b0t": np.ascontiguousarray(b0.reshape(4, 128).T),
        "b0h": np.ascontiguousarray((GCN_ALPHA * b0).reshape(4, 128).T),
        "bl0t": np.ascontiguousarray(np.asarray(inputs["bl0"], f32).reshape(4, 128).T),
        "bl1t": np.ascontiguousarray(np.asarray(inputs["bl1"], f32).reshape(4, 128).T),
        "bl2t": np.ascontiguousarray(np.asarray(inputs["bl2"], f32).reshape(4, 128).T),
        "bfin": np.ascontiguousarray(blast.reshape(C, 1)),
        "idf": np.eye(128, dtype=f32),
        "idb": np.eye(128, dtype=bfnp),
        "idq": idq,
    }
    return m


def _shard_maps(inputs, n_tiles, n_cores):
    feats = np.asarray(inputs["features"])
    lab = np.asarray(inputs["label_emb"])
    fb = feats.astype(bfnp)
    lb = lab.astype(bfnp)
    npc = n_tiles * T
    wmap = _prep_weights(inputs, n_tiles)
    maps = []
    for core in range(n_cores):
        sl = slice(core * npc, (core + 1) * npc)
        fsh = fb[:, sl, :]  # [H, npc, F]
        fN = np.ascontiguousarray(
            fsh.reshape(H, n_tiles, NS, SUB, F).transpose(1, 3, 0, 2, 4)
        )
        fT = np.ascontiguousarray(
            fsh.reshape(H, n_tiles, T, 2, 128).transpose(1, 4, 0, 3, 2)
        )
        eT = np.ascontiguousarray(lb[sl].reshape(n_tiles, T, C).transpose(0, 2, 1))
        m = dict(wmap)
        m["featN"] = fN
        m["featT"] = fT
        m["embT"] = eT
        maps.append(m)
    return maps


_CACHE = {}


def _get_nc(n_tiles, ba, a_out, a_lab, repeat=1):
    key = (n_tiles, repeat, round(float(ba), 8), round(float(a_out), 8), round(float(a_lab), 8))
    if key not in _CACHE:
        _CACHE[key] = build(n_tiles, float(ba), float(a_out), float(a_lab), repeat)
    return _CACHE[key]


def kernel(**inputs) -> np.ndarray:
    ba = float(np.asarray(inputs["ba"]))
    a_out = float(np.asarray(inputs["a_out"]))
    a_lab = float(np.asarray(inputs["a_lab"]))
    nc = _get_nc(NT_FULL, ba, a_out, a_lab)
    maps = _shard_maps(inputs, NT_FULL, N_CORES)
    res = run_bass_kernel_spmd(nc, maps, list(range(N_CORES)))
    outs = []
    for i in range(N_CORES):
        o = np.asarray(res.results[i]["outT"], np.float32)  # [nt, SUB, NS, C]
        outs.append(o.transpose(0, 2, 1, 3).reshape(-1, C))
    return np.concatenate(outs, axis=0)

